# revision 26
# baseline (speedup 1.0000x reference)
"""Trainium2 Bass kernel for the pairwise-score attention + gated MLP encoding.

Computation (per batch element b, p=1024 tokens, d=256 features):
    A[i,j]  = wa.P_i + wb.P_j + (P_i*wc).P_j
    itr     = softmax_j(A) @ P
    cat     = [P, itr]
    z       = tanh(cat@w1+b1); r = sigmoid(cat@w2+b2); f = sigmoid(cat@w3+b3)
    out     = r*P + f*z
Sharding: data-parallel over batch across 8 NeuronCores (4 batch el / core).

v4 design (fp8-DoubleRow scores/attention, bf16 MLP, 3-deep pipeline):
  - P loaded via gpsimd casting DMAs: natural bf16 in 4 chunk-pair tiles
    (Pnb, so PE transposes start as chunks land) and natural fp8e4 (Pn8).
  - P^T (bf16, PTb) via paired PE transposes (1 cyc/row) evacuated by DVE.
  - Scores^T[j,i] (wa-term cancels in softmax; wb-term folded into the exp
    bias) via fp8e4 DoubleRow matmuls (contract 256/pass, 0.5 cyc/row):
    stationary PT8, moving PcT8 = wc*PTb.  PT8/PcT8 produced on GPSIMD
    (slack engine) one full iteration ahead; batch 0 preps on DVE.
  - exp on ACT straight from 2-bank PSUM, fp8e4 output, sb bias (sb = P.wb
    via DVE mul + row-reduce).
  - Softmax denominator (ones8 stationary) and itr^T numerator (Pn8 pair
    stationary) also fp8 DoubleRow over expST8.  DVE normalizes with
    reciprocal + mul into bf16 itrT.
  - MLP transposed in bf16, biases as ACT per-partition biases; sigmoid via
    0.5+0.5*tanh(x/2) (single ACT table set).
  - Gating computes o' = 2*out in bf16 tensor_tensor ops; the 0.5 rides the
    PSUM-evacuation tensor_scalar_mul; output stays bf16 through the store
    (DRAM out is bf16, upcast on host).
  - 3-deep software pipeline: iteration b emits attn/norm/mlp/gate of b,
    out of b-1, transposes+prep of b+2, and scores of b+1 interleaved into
    b's MLP psum groups so the in-order ACT stream (6 acts + 8 exps per
    batch) always has producers ahead and the PE never re-triggers the HAM
    idle throttle.
"""

import os
import sys

if "/opt/trn_rl_repo" not in sys.path:
    sys.path.insert(0, "/opt/trn_rl_repo")

import numpy as np

import concourse.bass as bass
import concourse.mybir as mybir
import concourse.tile as tile
from concourse import bacc
from concourse.bass_utils import run_bass_kernel_spmd
from concourse.masks import make_identity

F32 = mybir.dt.float32
BF16 = mybir.dt.bfloat16
FP8 = mybir.dt.float8e4
AF = mybir.ActivationFunctionType
ALU = mybir.AluOpType
AXX = mybir.AxisListType
DR = mybir.MatmulPerfMode.DoubleRow

B, PLEN, D = 32, 1024, 256
N_CORES = 8
B_LOC = B // N_CORES  # batch elements per core

NJ = PLEN // 128  # 8 token chunks of 128
ND = D // 128     # 2 feature chunks of 128
NPAIR = NJ // 2   # 4 token chunk-pairs (DoubleRow contraction)


def _emit(ctx, tc, P_in, w_att, w_mlp, b_mlp, out):
    nc = tc.nc
    ts = bass.ts
    ds = bass.ds

    const = ctx.enter_context(tc.tile_pool(name="const", bufs=1))
    pload = ctx.enter_context(tc.tile_pool(name="pload", bufs=3))
    ptp = ctx.enter_context(tc.tile_pool(name="ptp", bufs=3))
    pexp = ctx.enter_context(tc.tile_pool(name="pexp", bufs=2))
    pitr = ctx.enter_context(tc.tile_pool(name="pitr", bufs=2))
    pmlp = ctx.enter_context(tc.tile_pool(name="pmlp", bufs=2))
    pout = ctx.enter_context(tc.tile_pool(name="pout", bufs=2))
    ps_big = ctx.enter_context(tc.tile_pool(name="ps_big", bufs=3, space="PSUM"))
    ps_t2 = ctx.enter_context(tc.tile_pool(name="ps_t2", bufs=2, space="PSUM"))

    # ---- batch loads: bf16 in 4 chunk-pair tiles + fp8 whole ----
    def phase_load(b):
        pnb = [pload.tile([128, 2, 256], BF16, tag=f"pnb{k}", name=f"pnb{k}_{b}")
               for k in range(4)]
        for k in range(4):
            src = P_in[b, ds(k * 256, 256), :].rearrange("(jc p) d -> p jc d",
                                                         p=128)
            nc.gpsimd.dma_start(out=pnb[k], in_=src)
        pn8 = pload.tile([128, NJ, 256], FP8, tag="pn8", name=f"pn8_{b}")
        nc.gpsimd.dma_start(
            out=pn8, in_=P_in[b, :, :].rearrange("(jc p) d -> p jc d", p=128))
        return pnb, pn8

    ld0 = phase_load(0)

    # ---- constants / weights (once per core; order = gpsimd queue order) ----
    wc_sb = []
    for dc in range(ND):
        wc = const.tile([128, 1], F32, tag=f"wc{dc}")
        nc.gpsimd.dma_start(out=wc,
                            in_=w_att[ds(2 * D + dc * 128, 128)].unsqueeze(1))
        wc_sb.append(wc)
    # wb broadcast to all partitions, repeated per chunk-pair: [128,2,256]
    _wbs = w_att[ds(D, D)]
    wbb2 = const.tile([128, 2, 256], BF16)
    nc.gpsimd.dma_start(
        out=wbb2,
        in_=bass.AP(tensor=_wbs.tensor, offset=_wbs.offset,
                    ap=[[0, 128], [0, 2]] + list(_wbs.ap)),
    )
    b_sb = []  # b_sb[wi][dc]
    for wi in range(3):
        chunks = []
        for dc in range(ND):
            bt = const.tile([128, 1], F32, tag=f"b{wi}{dc}")
            nc.gpsimd.dma_start(out=bt,
                                in_=b_mlp[wi][ds(dc * 128, 128)].unsqueeze(1))
            if wi > 0:
                bh = const.tile([128, 1], F32, tag=f"bh{wi}{dc}")
                nc.scalar.mul(out=bh, in_=bt, mul=0.5)
                bt = bh
            chunks.append(bt)
        b_sb.append(chunks)

    ident = const.tile([128, 128], F32)
    make_identity(nc, ident)
    identb = const.tile([128, 128], BF16)
    nc.vector.tensor_copy(out=identb, in_=ident)
    ones_f = const.tile([128, 256], F32)
    nc.vector.memset(ones_f, 1.0)
    ones8 = const.tile([128, 2, 128], FP8)
    nc.vector.tensor_copy(out=ones8, in_=ones_f.rearrange("p (t m) -> p t m", t=2))

    lds = {0: ld0, 1: phase_load(1)}

    # MLP weights last on the gpsimd queue (needed only at mlp(0))
    w_sb = []
    for wi in range(3):
        wt = const.tile([128, 4, D], BF16, tag=f"w{wi}")
        nc.gpsimd.dma_start(
            out=wt, in_=w_mlp[wi].rearrange("(kc k) d -> k kc d", k=128))
        w_sb.append(wt)

    # ---- per-batch-element phases ----
    def alloc_ptb(b):
        return ptp.tile([128, ND, PLEN], BF16, tag="ptb", name=f"ptb{b}")

    def emit_inT_quad(b, ptb, pnb, dc, q):
        # transpose token chunks 4q..4q+3 (tokens 512*q..512*q+511), chunk dc
        pst = ps_t2.tile([128, 512], BF16, tag="pst", name="psti")
        for h in range(4):
            nc.tensor.transpose(pst[:, ts(h, 128)],
                                pnb[2 * q + h // 2][:, h % 2, ts(dc, 128)],
                                identb)
        nc.scalar.copy(out=ptb[:, dc, ts(q, 512)], in_=pst)

    def phase_inT(b, pnb):
        ptb = alloc_ptb(b)
        for dc in range(ND):
            for q in range(2):
                emit_inT_quad(b, ptb, pnb, dc, q)
        return ptb

    def prep_scores(b, ptb):
        pt8 = ptp.tile([128, ND, PLEN], FP8, tag="pt8", name=f"pt8{b}")
        nc.vector.tensor_copy(out=pt8, in_=ptb)
        pct8 = ptp.tile([128, ND, PLEN], FP8, tag="pct8", name=f"pct8{b}")
        for dc in range(ND):
            nc.vector.tensor_scalar_mul(out=pct8[:, dc, :], in0=ptb[:, dc, :],
                                        scalar1=wc_sb[dc])
        return pt8, pct8

    def prep_sb(b, pnb):
        # sb[j] = P_j . wb : DVE mul + row-reduce per chunk-pair tile
        sbc = []
        for k in range(4):
            scr = ptp.tile([128, 2, 256], BF16, tag=f"scr{k}", name=f"scr{k}_{b}")
            nc.vector.tensor_mul(out=scr, in0=pnb[k], in1=wbb2)
            sc = ptp.tile([128, 2], F32, tag=f"sbc{k}", name=f"sbc{k}_{b}")
            nc.vector.reduce_sum(out=sc, in_=scr, axis=AXX.X)
            sbc.append(sc)
        return sbc

    def make_es(b):
        return [pexp.tile([128, 2, PLEN], FP8, tag=f"es{pr}", name=f"es{pr}_{b}")
                for pr in range(NPAIR)]

    def emit_score_jc(b, jc, pt8, pct8, sbc, es):
        pss = ps_big.tile([128, 1024], F32, tag="big", name=f"pss{jc}")
        for ic2 in range(2):
            nc.tensor.matmul(pss[:, ts(ic2, 512)],
                             pt8[:, :, ts(jc, 128)],
                             pct8[:, :, ts(ic2, 512)],
                             start=True, stop=True, perf_mode=DR)
        nc.scalar.activation(out=es[jc // 2][:, jc % 2, :], in_=pss,
                             func=AF.Exp, bias=sbc[jc // 2][:, ds(jc % 2, 1)],
                             scale=1.0)

    def phase_attn(b, pn8, es):
        # denominator + itr numerators; psd/pit0 accumulations stream by
        # chunk-pair so the PE starts as soon as the first exps land instead
        # of waiting for the whole exp chain (matters for batch 0 and HAM)
        psd = ps_big.tile([128, 1024], F32, tag="big", name="psd")
        pit0 = ps_big.tile([128, 1024], F32, tag="big", name="pit0")
        for pr in range(NPAIR):
            for ic2 in range(2):
                nc.tensor.matmul(psd[:, ts(ic2, 512)], ones8,
                                 es[pr][:, :, ts(ic2, 512)],
                                 start=(pr == 0), stop=(pr == NPAIR - 1),
                                 perf_mode=DR)
            for ic2 in range(2):
                nc.tensor.matmul(pit0[:, ts(ic2, 512)],
                                 pn8[:, ds(2 * pr, 2), ts(0, 128)],
                                 es[pr][:, :, ts(ic2, 512)],
                                 start=(pr == 0), stop=(pr == NPAIR - 1),
                                 perf_mode=DR)
        pit1 = ps_big.tile([128, 1024], F32, tag="big", name="pit1")
        for pr in range(NPAIR):
            for ic2 in range(2):
                nc.tensor.matmul(pit1[:, ts(ic2, 512)],
                                 pn8[:, ds(2 * pr, 2), ts(1, 128)],
                                 es[pr][:, :, ts(ic2, 512)],
                                 start=(pr == 0), stop=(pr == NPAIR - 1),
                                 perf_mode=DR)
        return psd, [pit0, pit1]

    def phase_norm(b, psd, pits):
        recipb = pitr.tile([128, PLEN], F32, tag="recipb", name=f"rec{b}")
        nc.vector.reciprocal_approx_fast(out=recipb, in_=psd)
        itrT = []
        for dc in range(ND):
            it = pitr.tile([128, PLEN], BF16, tag=f"it{dc}", name=f"itrT{dc}_{b}")
            nc.vector.tensor_mul(out=it, in0=pits[dc], in1=recipb)
            itrT.append(it)
        return itrT

    def emit_mlp_group(b, dc, wi, ptb, itrT):
        catT = [ptb[:, 0, :], ptb[:, 1, :], itrT[0], itrT[1]]
        psm = ps_big.tile([128, 1024], F32, tag="big", name=f"psm{dc}{wi}")
        for pc in range(2):
            for kc in range(4):
                nc.tensor.matmul(
                    psm[:, ts(pc, 512)],
                    w_sb[wi][:, kc, ts(dc, 128)],
                    catT[kc][:, ts(pc, 512)],
                    start=(kc == 0), stop=(kc == 3),
                )
        t = pmlp.tile([128, PLEN], BF16, tag=f"act{wi}", name=f"a{wi}d{dc}")
        if wi == 0:
            nc.scalar.activation(out=t, in_=psm, func=AF.Tanh,
                                 bias=b_sb[0][dc], scale=1.0)
        else:
            nc.scalar.activation(out=t, in_=psm, func=AF.Tanh,
                                 bias=b_sb[wi][dc], scale=0.5)
        return t

    def emit_gate_dc(b, dc, ptb, acts_dc, split=False):
        # o' = 2*out = (t2+1)*P + (t3+1)*z, all-bf16 tensor_tensor (2x DVE)
        z_t, t2, t3 = acts_dc
        o = pmlp.tile([128, PLEN], BF16, tag=f"oT{dc}", name=f"oT{dc}_{b}")
        m1 = pmlp.tile([128, PLEN], BF16, tag="m1", name="m1", bufs=2)
        m2 = pmlp.tile([128, PLEN], BF16, tag="m2", name="m2", bufs=2)
        slices = [ts(pc, 512) for pc in range(2)] if split else [ts(0, 1024)]
        for sl in slices:
            p_sl = ptb[:, dc, sl]
            nc.vector.tensor_mul(out=m1[:, sl], in0=t2[:, sl], in1=p_sl)
            nc.vector.tensor_add(out=m1[:, sl], in0=m1[:, sl], in1=p_sl)
            nc.vector.tensor_mul(out=m2[:, sl], in0=t3[:, sl], in1=z_t[:, sl])
            nc.vector.tensor_add(out=m2[:, sl], in0=m2[:, sl], in1=z_t[:, sl])
            nc.vector.tensor_add(out=o[:, sl], in0=m1[:, sl], in1=m2[:, sl])
        return o

    def emit_out_quad(b, oT, q):
        for q in [q]:
            pst = ps_t2.tile([128, 512], BF16, tag="pst", name="psto")
            for h in range(2):
                nc.tensor.transpose(pst[:, ts(2 * h, 128)],
                                    oT[0][:, ts(2 * q + h, 128)], identb)
                nc.tensor.transpose(pst[:, ts(2 * h + 1, 128)],
                                    oT[1][:, ts(2 * q + h, 128)], identb)
            onat = pout.tile([128, 2, D], BF16, tag=f"on{q}", name=f"onat{q}")
            nc.scalar.mul(out=onat, in_=pst, mul=0.5)
            nc.sync.dma_start(
                out=out[b, ds(q * 256, 256), :].rearrange(
                    "(p2 p) d -> p p2 d", p=128),
                in_=onat)

    def phase_out(b, oT):
        for q in range(4):
            emit_out_quad(b, oT, q)

    # scores jc's of b+1 interleaved between MLP psum groups of b
    # (group order: (dc,wi) = 00,01,02,10,11,12; gate dc emitted when done)
    MLP_ORDER = [(0, 0), (0, 1), (0, 2), (1, 0), (1, 1), (1, 2)]
    JC_BEFORE = [[], [0, 1], [2, 3], [4, 5], [6], [7]]

    # ---- prologue ----
    ptb0 = phase_inT(0, lds[0][0])
    pt8_0, pct8_0 = prep_scores(0, ptb0)
    sbc_0 = prep_sb(0, lds[0][0])
    prep = {0: (pt8_0, pct8_0, sbc_0)}
    ptbs = {0: ptb0, 1: alloc_ptb(1)}
    es = make_es(0)
    # scores(0) interleaved with inT(1) so the PE stays busy while ACT
    # drains the serial exp(0) chain
    inT1 = [(dc, q) for dc in range(ND) for q in range(2)]
    SC0 = [[0, 1, 2], [3], [4, 5], [6], [7]]
    for step in range(4):
        for jc in SC0[step]:
            emit_score_jc(0, jc, *prep[0], es)
        dc, q = inT1[step]
        emit_inT_quad(1, ptbs[1], lds[1][0], dc, q)
    for jc in SC0[4]:
        emit_score_jc(0, jc, *prep[0], es)
    prep[1] = (*prep_scores(1, ptbs[1]), prep_sb(1, lds[1][0]))

    # ---- main loop ----
    oT_prev = None
    for b in range(B_LOC):
        last = b + 1 >= B_LOC
        if b + 2 < B_LOC:
            lds[b + 2] = phase_load(b + 2)
        psd, pits = phase_attn(b, lds[b][1], es)
        itrT = phase_norm(b, psd, pits)
        if b + 2 < B_LOC:
            ptbs[b + 2] = phase_inT(b + 2, lds[b + 2][0])
        if oT_prev is not None and not last:
            phase_out(b - 1, oT_prev)
        if not last:
            es_n = make_es(b + 1)
        acts = [[None] * 3, [None] * 3]
        oT = [None, None]
        for gi, (dc, wi) in enumerate(MLP_ORDER):
            if not last:
                for jc in JC_BEFORE[gi]:
                    emit_score_jc(b + 1, jc, *prep[b + 1], es_n)
            acts[dc][wi] = emit_mlp_group(b, dc, wi, ptbs[b], itrT)
            if last and oT_prev is not None and gi < 4:
                emit_out_quad(b - 1, oT_prev, gi)
            if wi == 2:
                oT[dc] = emit_gate_dc(b, dc, ptbs[b], acts[dc],
                                      split=last)
            if gi == 2 and b + 2 < B_LOC:
                ps2 = prep_scores(b + 2, ptbs[b + 2])
            if gi == 4 and b + 2 < B_LOC:
                prep[b + 2] = (*ps2, prep_sb(b + 2, lds[b + 2][0]))
        oT_prev = oT
        if not last:
            es = es_n
    phase_out(B_LOC - 1, oT_prev)


_NC_CACHE = {}


def _build():
    if "nc" in _NC_CACHE:
        return _NC_CACHE["nc"]
    nc = bacc.Bacc("TRN2", target_bir_lowering=False, debug=False,
                   num_devices=N_CORES)
    P_in = nc.dram_tensor("p_in", [B_LOC, PLEN, D], F32, kind="ExternalInput").ap()
    w_att = nc.dram_tensor("w_att", [3 * D], F32, kind="ExternalInput").ap()
    w_mlp = [nc.dram_tensor(f"w{i}", [2 * D, D], F32, kind="ExternalInput").ap()
             for i in (1, 2, 3)]
    b_mlp = [nc.dram_tensor(f"b{i}", [D], F32, kind="ExternalInput").ap()
             for i in (1, 2, 3)]
    out = nc.dram_tensor("out", [B_LOC, PLEN, D], BF16,
                         kind="ExternalOutput").ap()

    from contextlib import ExitStack

    with tile.TileContext(nc) as tc, ExitStack() as ctx:
        _emit(ctx, tc, P_in, w_att, w_mlp, b_mlp, out)
    nc.compile()
    _NC_CACHE["nc"] = nc
    return nc


def run(inputs, trace=False, tmpdir=None):
    nc = _build()
    P = np.ascontiguousarray(np.asarray(inputs["P"], dtype=np.float32))
    shared = {
        "w_att": np.ascontiguousarray(np.asarray(inputs["w_itr_att"], np.float32)),
        "w1": np.ascontiguousarray(np.asarray(inputs["w1"], np.float32)),
        "w2": np.ascontiguousarray(np.asarray(inputs["w2"], np.float32)),
        "w3": np.ascontiguousarray(np.asarray(inputs["w3"], np.float32)),
        "b1": np.ascontiguousarray(np.asarray(inputs["b1"], np.float32)),
        "b2": np.ascontiguousarray(np.asarray(inputs["b2"], np.float32)),
        "b3": np.ascontiguousarray(np.asarray(inputs["b3"], np.float32)),
    }
    in_maps = [
        {"p_in": P[c * B_LOC : (c + 1) * B_LOC], **shared} for c in range(N_CORES)
    ]
    res = run_bass_kernel_spmd(nc, in_maps, list(range(N_CORES)), trace=trace,
                               tmpdir=tmpdir)
    full = np.concatenate(
        [np.asarray(res.results[c]["out"]).astype(np.float32)
         for c in range(N_CORES)], axis=0)
    return full, res


def kernel(**inputs):
    full, _ = run(inputs)
    return full


# revision 30
# speedup vs baseline: 1.0040x; 1.0040x over previous
"""Trainium2 Bass kernel for the pairwise-score attention + gated MLP encoding.

Computation (per batch element b, p=1024 tokens, d=256 features):
    A[i,j]  = wa.P_i + wb.P_j + (P_i*wc).P_j
    itr     = softmax_j(A) @ P
    cat     = [P, itr]
    z       = tanh(cat@w1+b1); r = sigmoid(cat@w2+b2); f = sigmoid(cat@w3+b3)
    out     = r*P + f*z
Sharding: data-parallel over batch across 8 NeuronCores (4 batch el / core).

v4 design (fp8-DoubleRow scores/attention, bf16 MLP, 3-deep pipeline):
  - P loaded via gpsimd casting DMAs: natural bf16 in 4 chunk-pair tiles
    (Pnb, so PE transposes start as chunks land) and natural fp8e4 (Pn8).
  - P^T (bf16, PTb) via paired PE transposes (1 cyc/row) evacuated by DVE.
  - Scores^T[j,i] (wa-term cancels in softmax; wb-term folded into the exp
    bias) via fp8e4 DoubleRow matmuls (contract 256/pass, 0.5 cyc/row):
    stationary PT8, moving PcT8 = wc*PTb.  PT8/PcT8 produced on GPSIMD
    (slack engine) one full iteration ahead; batch 0 preps on DVE.
  - exp on ACT straight from 2-bank PSUM, fp8e4 output, sb bias (sb = P.wb
    via DVE mul + row-reduce).
  - Softmax denominator (ones8 stationary) and itr^T numerator (Pn8 pair
    stationary) also fp8 DoubleRow over expST8.  DVE normalizes with
    reciprocal + mul into bf16 itrT.
  - MLP transposed in bf16, biases as ACT per-partition biases; sigmoid via
    0.5+0.5*tanh(x/2) (single ACT table set).
  - Gating computes o' = 2*out in bf16 tensor_tensor ops; the 0.5 rides the
    PSUM-evacuation tensor_scalar_mul; output stays bf16 through the store
    (DRAM out is bf16, upcast on host).
  - 3-deep software pipeline: iteration b emits attn/norm/mlp/gate of b,
    out of b-1, transposes+prep of b+2, and scores of b+1 interleaved into
    b's MLP psum groups so the in-order ACT stream (6 acts + 8 exps per
    batch) always has producers ahead and the PE never re-triggers the HAM
    idle throttle.
"""

import os
import sys

if "/opt/trn_rl_repo" not in sys.path:
    sys.path.insert(0, "/opt/trn_rl_repo")

import numpy as np

import concourse.bass as bass
import concourse.mybir as mybir
import concourse.tile as tile
from concourse import bacc
from concourse.bass_utils import run_bass_kernel_spmd
from concourse.masks import make_identity

F32 = mybir.dt.float32
BF16 = mybir.dt.bfloat16
FP8 = mybir.dt.float8e4
AF = mybir.ActivationFunctionType
ALU = mybir.AluOpType
AXX = mybir.AxisListType
DR = mybir.MatmulPerfMode.DoubleRow

B, PLEN, D = 32, 1024, 256
N_CORES = 8
B_LOC = B // N_CORES  # batch elements per core

NJ = PLEN // 128  # 8 token chunks of 128
ND = D // 128     # 2 feature chunks of 128
NPAIR = NJ // 2   # 4 token chunk-pairs (DoubleRow contraction)


def _emit(ctx, tc, P_in, w_att, w_mlp, b_mlp, out):
    nc = tc.nc
    ts = bass.ts
    ds = bass.ds

    const = ctx.enter_context(tc.tile_pool(name="const", bufs=1))
    pload = ctx.enter_context(tc.tile_pool(name="pload", bufs=3))
    ptp = ctx.enter_context(tc.tile_pool(name="ptp", bufs=3))
    pexp = ctx.enter_context(tc.tile_pool(name="pexp", bufs=2))
    pitr = ctx.enter_context(tc.tile_pool(name="pitr", bufs=2))
    pmlp = ctx.enter_context(tc.tile_pool(name="pmlp", bufs=2))
    pout = ctx.enter_context(tc.tile_pool(name="pout", bufs=2))
    ps_big = ctx.enter_context(tc.tile_pool(name="ps_big", bufs=3, space="PSUM"))
    ps_t2 = ctx.enter_context(tc.tile_pool(name="ps_t2", bufs=2, space="PSUM"))

    # ---- batch loads: bf16 in 4 chunk-pair tiles + fp8 whole ----
    def phase_load(b):
        pnb = [pload.tile([128, 2, 256], BF16, tag=f"pnb{k}", name=f"pnb{k}_{b}")
               for k in range(4)]
        for k in range(4):
            src = P_in[b, ds(k * 256, 256), :].rearrange("(jc p) d -> p jc d",
                                                         p=128)
            nc.gpsimd.dma_start(out=pnb[k], in_=src)
        pn8 = pload.tile([128, NJ, 256], FP8, tag="pn8", name=f"pn8_{b}")
        nc.gpsimd.dma_start(
            out=pn8, in_=P_in[b, :, :].rearrange("(jc p) d -> p jc d", p=128))
        return pnb, pn8

    ld0 = phase_load(0)

    # ---- constants / weights (once per core; order = gpsimd queue order) ----
    wc_sb = []
    for dc in range(ND):
        wc = const.tile([128, 1], F32, tag=f"wc{dc}")
        nc.gpsimd.dma_start(out=wc,
                            in_=w_att[ds(2 * D + dc * 128, 128)].unsqueeze(1))
        wc_sb.append(wc)
    # wb broadcast to all partitions, repeated per chunk-pair: [128,2,256]
    _wbs = w_att[ds(D, D)]
    wbb2 = const.tile([128, 2, 256], BF16)
    nc.gpsimd.dma_start(
        out=wbb2,
        in_=bass.AP(tensor=_wbs.tensor, offset=_wbs.offset,
                    ap=[[0, 128], [0, 2]] + list(_wbs.ap)),
    )
    b_sb = []  # b_sb[wi][dc]
    for wi in range(3):
        chunks = []
        for dc in range(ND):
            bt = const.tile([128, 1], F32, tag=f"b{wi}{dc}")
            nc.gpsimd.dma_start(out=bt,
                                in_=b_mlp[wi][ds(dc * 128, 128)].unsqueeze(1))
            if wi > 0:
                bh = const.tile([128, 1], F32, tag=f"bh{wi}{dc}")
                nc.scalar.mul(out=bh, in_=bt, mul=0.5)
                bt = bh
            chunks.append(bt)
        b_sb.append(chunks)

    ident = const.tile([128, 128], F32)
    make_identity(nc, ident)
    identb = const.tile([128, 128], BF16)
    nc.vector.tensor_copy(out=identb, in_=ident)
    ones_f = const.tile([128, 256], F32)
    nc.vector.memset(ones_f, 1.0)
    ones8 = const.tile([128, 2, 128], FP8)
    nc.vector.tensor_copy(out=ones8, in_=ones_f.rearrange("p (t m) -> p t m", t=2))

    lds = {0: ld0, 1: phase_load(1)}

    # MLP weights last on the gpsimd queue (needed only at mlp(0))
    w_sb = []
    for wi in range(3):
        wt = const.tile([128, 4, D], BF16, tag=f"w{wi}")
        nc.gpsimd.dma_start(
            out=wt, in_=w_mlp[wi].rearrange("(kc k) d -> k kc d", k=128))
        w_sb.append(wt)

    # ---- per-batch-element phases ----
    def alloc_ptb(b):
        return ptp.tile([128, ND, PLEN], BF16, tag="ptb", name=f"ptb{b}")

    def emit_inT_quad(b, ptb, pnb, dc, q, dve_evac=False):
        # transpose token chunks 4q..4q+3 (tokens 512*q..512*q+511), chunk dc
        pst = ps_t2.tile([128, 512], BF16, tag="pst", name="psti")
        for h in range(4):
            nc.tensor.transpose(pst[:, ts(h, 128)],
                                pnb[2 * q + h // 2][:, h % 2, ts(dc, 128)],
                                identb)
        # early-pipeline evacs go on DVE so they never sit inside the ACT
        # exp chain (head) or ahead of the first MLP activation (iter 0)
        if dve_evac:
            nc.vector.tensor_copy(out=ptb[:, dc, ts(q, 512)], in_=pst)
        else:
            nc.scalar.copy(out=ptb[:, dc, ts(q, 512)], in_=pst)

    def phase_inT(b, pnb, dve_evac=False):
        ptb = alloc_ptb(b)
        for dc in range(ND):
            for q in range(2):
                emit_inT_quad(b, ptb, pnb, dc, q, dve_evac)
        return ptb

    def prep_scores(b, ptb):
        pt8 = ptp.tile([128, ND, PLEN], FP8, tag="pt8", name=f"pt8{b}")
        nc.vector.tensor_copy(out=pt8, in_=ptb)
        pct8 = ptp.tile([128, ND, PLEN], FP8, tag="pct8", name=f"pct8{b}")
        for dc in range(ND):
            nc.vector.tensor_scalar_mul(out=pct8[:, dc, :], in0=ptb[:, dc, :],
                                        scalar1=wc_sb[dc])
        return pt8, pct8

    def prep_sb(b, pnb):
        # sb[j] = P_j . wb : DVE mul + row-reduce per chunk-pair tile
        sbc = []
        for k in range(4):
            scr = ptp.tile([128, 2, 256], BF16, tag=f"scr{k}", name=f"scr{k}_{b}")
            nc.vector.tensor_mul(out=scr, in0=pnb[k], in1=wbb2)
            sc = ptp.tile([128, 2], F32, tag=f"sbc{k}", name=f"sbc{k}_{b}")
            nc.vector.reduce_sum(out=sc, in_=scr, axis=AXX.X)
            sbc.append(sc)
        return sbc

    def make_es(b):
        return [pexp.tile([128, 2, PLEN], FP8, tag=f"es{pr}", name=f"es{pr}_{b}")
                for pr in range(NPAIR)]

    def emit_score_jc(b, jc, pt8, pct8, sbc, es):
        pss = ps_big.tile([128, 1024], F32, tag="big", name=f"pss{jc}")
        for ic2 in range(2):
            nc.tensor.matmul(pss[:, ts(ic2, 512)],
                             pt8[:, :, ts(jc, 128)],
                             pct8[:, :, ts(ic2, 512)],
                             start=True, stop=True, perf_mode=DR)
        nc.scalar.activation(out=es[jc // 2][:, jc % 2, :], in_=pss,
                             func=AF.Exp, bias=sbc[jc // 2][:, ds(jc % 2, 1)],
                             scale=1.0)

    def phase_attn(b, pn8, es):
        # denominator + itr numerators; psd/pit0 accumulations stream by
        # chunk-pair so the PE starts as soon as the first exps land instead
        # of waiting for the whole exp chain (matters for batch 0 and HAM)
        psd = ps_big.tile([128, 1024], F32, tag="big", name="psd")
        pit0 = ps_big.tile([128, 1024], F32, tag="big", name="pit0")
        for pr in range(NPAIR):
            for ic2 in range(2):
                nc.tensor.matmul(psd[:, ts(ic2, 512)], ones8,
                                 es[pr][:, :, ts(ic2, 512)],
                                 start=(pr == 0), stop=(pr == NPAIR - 1),
                                 perf_mode=DR)
            for ic2 in range(2):
                nc.tensor.matmul(pit0[:, ts(ic2, 512)],
                                 pn8[:, ds(2 * pr, 2), ts(0, 128)],
                                 es[pr][:, :, ts(ic2, 512)],
                                 start=(pr == 0), stop=(pr == NPAIR - 1),
                                 perf_mode=DR)
        pit1 = ps_big.tile([128, 1024], F32, tag="big", name="pit1")
        for pr in range(NPAIR):
            for ic2 in range(2):
                nc.tensor.matmul(pit1[:, ts(ic2, 512)],
                                 pn8[:, ds(2 * pr, 2), ts(1, 128)],
                                 es[pr][:, :, ts(ic2, 512)],
                                 start=(pr == 0), stop=(pr == NPAIR - 1),
                                 perf_mode=DR)
        return psd, [pit0, pit1]

    def phase_norm(b, psd, pits):
        recipb = pitr.tile([128, PLEN], F32, tag="recipb", name=f"rec{b}")
        nc.vector.reciprocal_approx_fast(out=recipb, in_=psd)
        itrT = []
        for dc in range(ND):
            it = pitr.tile([128, PLEN], BF16, tag=f"it{dc}", name=f"itrT{dc}_{b}")
            nc.vector.tensor_mul(out=it, in0=pits[dc], in1=recipb)
            itrT.append(it)
        return itrT

    def emit_mlp_group(b, dc, wi, ptb, itrT):
        catT = [ptb[:, 0, :], ptb[:, 1, :], itrT[0], itrT[1]]
        psm = ps_big.tile([128, 1024], F32, tag="big", name=f"psm{dc}{wi}")
        for pc in range(2):
            for kc in range(4):
                nc.tensor.matmul(
                    psm[:, ts(pc, 512)],
                    w_sb[wi][:, kc, ts(dc, 128)],
                    catT[kc][:, ts(pc, 512)],
                    start=(kc == 0), stop=(kc == 3),
                )
        t = pmlp.tile([128, PLEN], BF16, tag=f"act{wi}", name=f"a{wi}d{dc}")
        if wi == 0:
            nc.scalar.activation(out=t, in_=psm, func=AF.Tanh,
                                 bias=b_sb[0][dc], scale=1.0)
        else:
            nc.scalar.activation(out=t, in_=psm, func=AF.Tanh,
                                 bias=b_sb[wi][dc], scale=0.5)
        return t

    def emit_gate_dc(b, dc, ptb, acts_dc, split=False):
        # o' = 2*out = (t2+1)*P + (t3+1)*z, all-bf16 tensor_tensor (2x DVE)
        z_t, t2, t3 = acts_dc
        o = pmlp.tile([128, PLEN], BF16, tag=f"oT{dc}", name=f"oT{dc}_{b}")
        m1 = pmlp.tile([128, PLEN], BF16, tag="m1", name="m1", bufs=2)
        m2 = pmlp.tile([128, PLEN], BF16, tag="m2", name="m2", bufs=2)
        slices = [ts(pc, 512) for pc in range(2)] if split else [ts(0, 1024)]
        for sl in slices:
            p_sl = ptb[:, dc, sl]
            nc.vector.tensor_mul(out=m1[:, sl], in0=t2[:, sl], in1=p_sl)
            nc.vector.tensor_add(out=m1[:, sl], in0=m1[:, sl], in1=p_sl)
            nc.vector.tensor_mul(out=m2[:, sl], in0=t3[:, sl], in1=z_t[:, sl])
            nc.vector.tensor_add(out=m2[:, sl], in0=m2[:, sl], in1=z_t[:, sl])
            nc.vector.tensor_add(out=o[:, sl], in0=m1[:, sl], in1=m2[:, sl])
        return o

    def phase_out(b, oT):
        for q in range(4):
            pst = ps_t2.tile([128, 512], BF16, tag="pst", name="psto")
            for h in range(2):
                nc.tensor.transpose(pst[:, ts(2 * h, 128)],
                                    oT[0][:, ts(2 * q + h, 128)], identb)
                nc.tensor.transpose(pst[:, ts(2 * h + 1, 128)],
                                    oT[1][:, ts(2 * q + h, 128)], identb)
            onat = pout.tile([128, 2, D], BF16, tag=f"on{q}", name=f"onat{q}")
            nc.vector.tensor_scalar_mul(out=onat, in0=pst, scalar1=0.5)
            nc.sync.dma_start(
                out=out[b, ds(q * 256, 256), :].rearrange(
                    "(p2 p) d -> p p2 d", p=128),
                in_=onat)

    # scores jc's of b+1 interleaved between MLP psum groups of b
    # (group order: (dc,wi) = 00,01,02,10,11,12; gate dc emitted when done)
    MLP_ORDER = [(0, 0), (0, 1), (0, 2), (1, 0), (1, 1), (1, 2)]
    JC_BEFORE = [[], [0, 1], [2, 3], [4, 5], [6], [7]]

    # ---- prologue ----
    ptb0 = phase_inT(0, lds[0][0])
    pt8_0, pct8_0 = prep_scores(0, ptb0)
    sbc_0 = prep_sb(0, lds[0][0])
    prep = {0: (pt8_0, pct8_0, sbc_0)}
    ptbs = {0: ptb0, 1: alloc_ptb(1)}
    es = make_es(0)
    # scores(0) interleaved with inT(1) so the PE stays busy while ACT
    # drains the serial exp(0) chain
    inT1 = [(dc, q) for dc in range(ND) for q in range(2)]
    SC0 = [[0, 1, 2], [3], [4, 5], [6], [7]]
    for step in range(4):
        for jc in SC0[step]:
            emit_score_jc(0, jc, *prep[0], es)
        dc, q = inT1[step]
        emit_inT_quad(1, ptbs[1], lds[1][0], dc, q, dve_evac=True)
    for jc in SC0[4]:
        emit_score_jc(0, jc, *prep[0], es)
    prep[1] = (*prep_scores(1, ptbs[1]), prep_sb(1, lds[1][0]))

    # ---- main loop ----
    oT_prev = None
    for b in range(B_LOC):
        last = b + 1 >= B_LOC
        if b + 2 < B_LOC:
            lds[b + 2] = phase_load(b + 2)
        psd, pits = phase_attn(b, lds[b][1], es)
        itrT = phase_norm(b, psd, pits)
        if b + 2 < B_LOC:
            ptbs[b + 2] = phase_inT(b + 2, lds[b + 2][0], dve_evac=(b == 0))
        if oT_prev is not None:
            phase_out(b - 1, oT_prev)
        if not last:
            es_n = make_es(b + 1)
        acts = [[None] * 3, [None] * 3]
        oT = [None, None]
        for gi, (dc, wi) in enumerate(MLP_ORDER):
            if not last:
                for jc in JC_BEFORE[gi]:
                    emit_score_jc(b + 1, jc, *prep[b + 1], es_n)
            acts[dc][wi] = emit_mlp_group(b, dc, wi, ptbs[b], itrT)
            if wi == 2:
                oT[dc] = emit_gate_dc(b, dc, ptbs[b], acts[dc],
                                      split=last)
            if gi == 2 and b + 2 < B_LOC:
                ps2 = prep_scores(b + 2, ptbs[b + 2])
            if gi == 4 and b + 2 < B_LOC:
                prep[b + 2] = (*ps2, prep_sb(b + 2, lds[b + 2][0]))
        oT_prev = oT
        if not last:
            es = es_n
    phase_out(B_LOC - 1, oT_prev)


_NC_CACHE = {}


def _build():
    if "nc" in _NC_CACHE:
        return _NC_CACHE["nc"]
    nc = bacc.Bacc("TRN2", target_bir_lowering=False, debug=False,
                   num_devices=N_CORES)
    P_in = nc.dram_tensor("p_in", [B_LOC, PLEN, D], F32, kind="ExternalInput").ap()
    w_att = nc.dram_tensor("w_att", [3 * D], F32, kind="ExternalInput").ap()
    w_mlp = [nc.dram_tensor(f"w{i}", [2 * D, D], F32, kind="ExternalInput").ap()
             for i in (1, 2, 3)]
    b_mlp = [nc.dram_tensor(f"b{i}", [D], F32, kind="ExternalInput").ap()
             for i in (1, 2, 3)]
    out = nc.dram_tensor("out", [B_LOC, PLEN, D], BF16,
                         kind="ExternalOutput").ap()

    from contextlib import ExitStack

    with tile.TileContext(nc) as tc, ExitStack() as ctx:
        _emit(ctx, tc, P_in, w_att, w_mlp, b_mlp, out)
    nc.compile()
    _NC_CACHE["nc"] = nc
    return nc


def run(inputs, trace=False, tmpdir=None):
    nc = _build()
    P = np.ascontiguousarray(np.asarray(inputs["P"], dtype=np.float32))
    shared = {
        "w_att": np.ascontiguousarray(np.asarray(inputs["w_itr_att"], np.float32)),
        "w1": np.ascontiguousarray(np.asarray(inputs["w1"], np.float32)),
        "w2": np.ascontiguousarray(np.asarray(inputs["w2"], np.float32)),
        "w3": np.ascontiguousarray(np.asarray(inputs["w3"], np.float32)),
        "b1": np.ascontiguousarray(np.asarray(inputs["b1"], np.float32)),
        "b2": np.ascontiguousarray(np.asarray(inputs["b2"], np.float32)),
        "b3": np.ascontiguousarray(np.asarray(inputs["b3"], np.float32)),
    }
    in_maps = [
        {"p_in": P[c * B_LOC : (c + 1) * B_LOC], **shared} for c in range(N_CORES)
    ]
    res = run_bass_kernel_spmd(nc, in_maps, list(range(N_CORES)), trace=trace,
                               tmpdir=tmpdir)
    full = np.concatenate(
        [np.asarray(res.results[c]["out"]).astype(np.float32)
         for c in range(N_CORES)], axis=0)
    return full, res


def kernel(**inputs):
    full, _ = run(inputs)
    return full
